# revision 10
# baseline (speedup 1.0000x reference)
"""Trainium2 Bass kernel for nn_DRModel_52630529245544.

Data-parallel over users (B=128 -> 16/core x 8 cores).
Per core:
  - basket embedding gather via GPSIMD dma_gather (transpose=True, int16
    indices biased around the table midpoint so signed int16 spans the
    50002-row vocab; +128 positive-sentinel tail indices)
  - masked mean-pool via DVE reduce (padding row 0 is all zeros)
  - x-side GRU projection precomputed for all timesteps (bf16 matmuls)
  - sequential GRU over T=100 in hidden-transposed layout [H in
    partitions, batch in free dim]; sigmoid computed via tanh identity
    (0.5*tanh(x/2)+0.5, folded into weights) so the ACT engine never
    reloads activation tables inside the loop.
"""
import os
import sys
import time

sys.path.insert(0, "/opt/trn_rl_repo")
sys.path.insert(0, os.path.dirname(os.path.abspath(__file__)))

import numpy as np
import ml_dtypes

BF16 = ml_dtypes.bfloat16

# problem constants (hardcoded per spec)
B, T, K, H, V = 128, 100, 25, 256, 50002
NCORES = 8
BL = B // NCORES          # 16 users per core
F = H + 10                # 266
G3 = 3 * H                # 768
TB = 8                    # timesteps per gather tile
NB_FULL = 13              # ceil(104/8); T padded to 104
TP = TB * NB_FULL         # 104
NIDX = 128 * K            # 3200 real slots per tile
NPAD = NIDX + 128         # +128 positive-sentinel tail
GIW = NB_FULL * 128       # 1664 columns per m-chunk in GI'
VHALF = V // 2            # 25001

_CACHE = {}


def _build(nb, nsteps):
    from tc_common import TC2
    import concourse.bass as bass
    import concourse.bacc as bacc
    from concourse import mybir

    nc = bacc.Bacc("TRN2")
    f32, bf16, i32, i16 = (
        mybir.dt.float32, mybir.dt.bfloat16, mybir.dt.int32, mybir.dt.int16,
    )
    AF = mybir.ActivationFunctionType

    # ---- I/O ----
    tab = nc.declare_dram_parameter("tab", [V, H], bf16, isOutput=False)
    idxg = nc.declare_dram_parameter("idxg", [nb, 128, NPAD // 16], i16, isOutput=False)
    idxc = nc.declare_dram_parameter("idxc", [nb, 128, K], i32, isOutput=False)
    wih = nc.declare_dram_parameter("wih", [F, G3], f32, isOutput=False)
    whh = nc.declare_dram_parameter("whh", [H, G3], f32, isOutput=False)
    brz = nc.declare_dram_parameter("brz", [128, 4], f32, isOutput=False)
    bn = nc.declare_dram_parameter("bn", [128, 2], f32, isOutput=False)
    bhn = nc.declare_dram_parameter("bhn", [1, H], f32, isOutput=False)
    hourt = nc.declare_dram_parameter("hourt", [T, BL], f32, isOutput=False)
    dowt = nc.declare_dram_parameter("dowt", [T, BL], f32, isOutput=False)
    d2nt = nc.declare_dram_parameter("d2nt", [T, BL], f32, isOutput=False)
    lens = nc.declare_dram_parameter("lens", [BL, 1], f32, isOutput=False)
    iotain = nc.declare_dram_parameter("iotain", [BL, TP], f32, isOutput=False)
    identbf = nc.declare_dram_parameter("identbf", [128, 128], bf16, isOutput=False)
    identf = nc.declare_dram_parameter("identf", [128, 128], f32, isOutput=False)
    h0t = nc.declare_dram_parameter("h0t", [128, 2 * BL], f32, isOutput=False)
    dbg = os.environ.get("KDBG", "0") == "1"
    if dbg:
        dgi = nc.declare_dram_parameter("dgi", [128, 6 * nb * 128], f32, isOutput=True)
        dpb = nc.declare_dram_parameter("dpb", [nb, 128, 2 * 128], f32, isOutput=True)
        dmk = nc.declare_dram_parameter("dmk", [128, nb * TB * 2 * BL], f32, isOutput=True)
        drz = nc.declare_dram_parameter("drz", [128, TB * 4 * BL], f32, isOutput=True)
        dnr = nc.declare_dram_parameter("dnr", [128, TB * 2 * BL], f32, isOutput=True)
        ddr = nc.declare_dram_parameter("ddr", [128, TB * 2 * BL], f32, isOutput=True)
    outT = nc.declare_dram_parameter("outT", [128, T * 2 * BL], f32, isOutput=True)
    hlast = nc.declare_dram_parameter("hlast", [128, 2 * BL], f32, isOutput=True)

    W = 2 * BL  # 32: (hc, b) folded column width

    with TC2(nc) as tc, (
        tc.tile_pool(name="const", bufs=1)
    ) as const, (
        tc.tile_pool(name="embp", bufs=2)
    ) as embp, (
        tc.tile_pool(name="stage", bufs=3)
    ) as stage, (
        tc.tile_pool(name="psA", bufs=3, space="PSUM")
    ) as psA, (
        tc.tile_pool(name="psG", bufs=2, space="PSUM")
    ) as psG, (
        tc.tile_pool(name="hp", bufs=2)
    ) as hp, (
        tc.tile_pool(name="gt", bufs=2)
    ) as gt:
        # ---------- constants / weights ----------
        ibf = const.tile([128, 128], bf16)
        nc.sync.dma_start(out=ibf[:], in_=identbf[:])
        if32 = const.tile([128, 128], f32)
        nc.sync.dma_start(out=if32[:], in_=identf[:])
        ones1 = const.tile([1, 128], f32)
        nc.vector.memset(ones1[:], 1.0)
        ones16 = const.tile([1, BL], f32)
        nc.vector.memset(ones16[:], 1.0)
        zc = const.tile([128, 1], f32)
        nc.vector.memset(zc[:], 0.0)
        pih = const.tile([128, 1], f32)
        nc.vector.memset(pih[:], float(np.pi / 2))

        # W_ih^T tiles: k-chunks (128,128,10) x m-chunks(6): [128, 18*128] bf16
        wihsb = const.tile([128, 18 * 128], f32)
        for kc, (k0, kn) in enumerate(((0, 128), (128, 128), (256, 10))):
            for m in range(6):
                nc.sync.dma_start(
                    out=wihsb[:kn, (kc * 6 + m) * 128:(kc * 6 + m) * 128 + 128],
                    in_=wih[k0:k0 + kn, m * 128:(m + 1) * 128],
                )
        whhsb = const.tile([128, 12 * 128], f32)
        for kc in range(2):
            for m in range(6):
                nc.sync.dma_start(
                    out=whhsb[:, (m * 2 + kc) * 128:(m * 2 + kc) * 128 + 128],
                    in_=whh[kc * 128:(kc + 1) * 128, m * 128:(m + 1) * 128],
                )
        brzsb = const.tile([128, 4], f32)
        nc.sync.dma_start(out=brzsb[:], in_=brz[:])
        bnsb = const.tile([128, 2], f32)
        nc.sync.dma_start(out=bnsb[:], in_=bn[:])
        bhnsb = const.tile([1, H], f32)
        nc.sync.dma_start(out=bhnsb[:], in_=bhn[:])

        # ---------- extra features -> xt_extra [10, nb*128] bf16 ----------
        hsb = const.tile([T, BL], f32)
        nc.sync.dma_start(out=hsb[:], in_=hourt[:])
        dsb = const.tile([T, BL], f32)
        nc.sync.dma_start(out=dsb[:], in_=dowt[:])
        nsb = const.tile([T, BL], f32)
        nc.sync.dma_start(out=nsb[:], in_=d2nt[:])
        feat = const.tile([T, 10 * BL], f32)
        PI = float(np.pi)
        nc.scalar.activation(out=feat[:, 0 * BL:1 * BL], in_=hsb[:], func=AF.Sin,
                             scale=PI / 23.0, bias=zc[:T])
        nc.scalar.activation(out=feat[:, 1 * BL:2 * BL], in_=hsb[:], func=AF.Sin,
                             scale=-PI / 23.0, bias=pih[:T])
        nc.scalar.activation(out=feat[:, 2 * BL:3 * BL], in_=nsb[:], func=AF.Copy,
                             scale=0.2)
        for j in range(7):
            nc.vector.tensor_scalar(
                out=feat[:, (3 + j) * BL:(4 + j) * BL], in0=dsb[:],
                scalar1=float(j), scalar2=None, op0=mybir.AluOpType.is_equal,
            )
        xte = const.tile([10, nb * 128], f32)
        n_tcols = min(T, nb * TB)
        for fi in range(10):
            nc.sync.dma_start(
                out=xte[fi:fi + 1, :n_tcols * BL].rearrange("o (t b) -> o t b", b=BL),
                in_=feat[:n_tcols, fi * BL:(fi + 1) * BL],
            )

        # ---------- masks [128, nsteps*W] bf16 ----------
        iot = const.tile([BL, TP], f32)
        nc.sync.dma_start(out=iot[:], in_=iotain[:])
        lsb = const.tile([BL, 1], f32)
        nc.sync.dma_start(out=lsb[:], in_=lens[:])
        mrow = const.tile([BL, TP], f32)
        nc.vector.tensor_scalar(out=mrow[:], in0=iot[:], scalar1=lsb[:, 0:1],
                                scalar2=None, op0=mybir.AluOpType.is_lt)
        mps = psA.tile([TP, BL], f32, name="mps", tag="mps", bufs=1)
        nc.tensor.matmul(out=mps[:], lhsT=mrow[:], rhs=if32[:BL, :BL],
                         is_transpose=True, start=True, stop=True)
        mT = const.tile([TP, BL], f32)
        nc.scalar.activation(out=mT[:], in_=mps[:], func=AF.Copy)
        mrow2 = const.tile([1, nsteps * W], f32)
        _m = mrow2[:]
        for hc in range(2):
            dst = bass.AP(tensor=_m.tensor, offset=_m.offset + hc * BL,
                          ap=[_m.ap[0], [W, nsteps], [1, BL]])
            nc.sync.dma_start(out=dst, in_=mT[:nsteps, :])
        masks = const.tile([128, nsteps * W], bf16)
        c0 = 0
        while c0 < nsteps * W:
            cw = min(512, nsteps * W - c0)
            mbp = psA.tile([128, 512], f32, name="mbp", tag="mps", bufs=1)
            nc.tensor.matmul(out=mbp[:, :cw], lhsT=ones1[:], rhs=mrow2[:, c0:c0 + cw],
                             start=True, stop=True)
            nc.scalar.activation(out=masks[:, c0:c0 + cw], in_=mbp[:, :cw], func=AF.Copy)
            c0 += cw

        # ---------- GI' [128, 6 * nb*128] bf16 ----------
        gib = const.tile([128, 6 * nb * 128], f32)
        giw = nb * 128

        # ---------- output accumulator ----------
        osb = const.tile([128, T * W], f32)

        # ---------- stage A ----------
        def _bcast2(bass, tile_):
            a = tile_[:]
            return bass.AP(tensor=a.tensor, offset=a.offset,
                           ap=[a.ap[0], [0, 2], [1, 128]])

        def stageA(i):
            ncols = 128 if (i < nb - 1 or nb * TB <= T) else (T - (nb - 1) * TB) * BL
            it = embp.tile([128, NPAD // 16], i16, name="it", tag="it")
            nc.sync.dma_start(out=it[:], in_=idxg[i])
            embT = embp.tile([128, 2, NPAD], bf16, name="embT", tag="embT")
            nc.gpsimd.dma_gather(
                out_ap=embT[:], in_ap=tab[VHALF:, :], idxs_ap=it[:],
                num_idxs=NPAD, num_idxs_reg=NPAD, elem_size=H,
                transpose=True, single_packet=False,
            )
            red = stage.tile([128, 2, 128], f32, name="red", tag="red")
            nc.vector.reduce_sum(
                out=red[:],
                in_=embT[:, :, :NIDX].rearrange("p c (b k) -> p c b k", k=K),
                axis=mybir.AxisListType.X,
            )
            # counts -> reciprocal
            ict = stage.tile([128, K], i32, name="ict", tag="ict")
            nc.sync.dma_start(out=ict[:], in_=idxc[i])
            c1 = stage.tile([128, K], f32, name="c1", tag="c1")
            nc.vector.tensor_scalar(out=c1[:], in0=ict[:], scalar1=0.5,
                                    scalar2=None, op0=mybir.AluOpType.is_gt)
            cs = stage.tile([128, 1], f32, name="cs", tag="cs")
            nc.vector.reduce_sum(out=cs[:], in_=c1[:], axis=mybir.AxisListType.X)
            nc.vector.tensor_scalar(out=cs[:], in0=cs[:], scalar1=1.0,
                                    scalar2=None, op0=mybir.AluOpType.max)
            rc = stage.tile([128, 1], f32, name="rc", tag="rc")
            nc.vector.reciprocal(out=rc[:], in_=cs[:])
            rcp = psA.tile([1, 128], f32, name="rcp", tag="rcp", bufs=1)
            nc.tensor.matmul(out=rcp[:], lhsT=rc[:], rhs=if32[:],
                             is_transpose=True, start=True, stop=True)
            rcr = stage.tile([1, 128], f32, name="rcr", tag="rcr")
            nc.scalar.activation(out=rcr[:], in_=rcp[:], func=AF.Copy)
            rcb = psA.tile([128, 128], f32, name="rcb", tag="rcb", bufs=1)
            nc.tensor.matmul(out=rcb[:], lhsT=ones1[:], rhs=rcr[:],
                             start=True, stop=True)
            pbf = stage.tile([128, 2, 128], f32, name="pbf", tag="pbf")
            nc.vector.tensor_tensor(
                out=pbf[:], in0=red[:],
                in1=_bcast2(bass, rcb),
                op=mybir.AluOpType.mult,
            )
            if dbg:
                nc.sync.dma_start(out=dpb[i], in_=pbf[:].rearrange("p c x -> p (c x)"))
            # x-side matmuls
            for m in range(6):
                gp = psA.tile([128, 128], f32, name="gp", tag="gp", bufs=2)
                nc.tensor.matmul(out=gp[:, :ncols], lhsT=wihsb[:, m * 128:(m + 1) * 128],
                                 rhs=pbf[:, 0, :ncols], start=True, stop=False)
                nc.tensor.matmul(out=gp[:, :ncols], lhsT=wihsb[:, (6 + m) * 128:(7 + m) * 128],
                                 rhs=pbf[:, 1, :ncols], start=False, stop=False)
                nc.tensor.matmul(out=gp[:, :ncols], lhsT=wihsb[:10, (12 + m) * 128:(12 + m) * 128 + 128],
                                 rhs=xte[:10, i * 128:i * 128 + ncols], start=False, stop=True)
                bias_ap = brzsb[:, m:m + 1] if m < 4 else bnsb[:, m - 4:m - 3]
                nc.scalar.activation(out=gib[:, m * giw + i * 128: m * giw + i * 128 + ncols],
                                     in_=gp[:, :ncols], func=AF.Identity, bias=bias_ap)

        # ---------- GRU ----------
        h_cur = hp.tile([128, W], f32, name="h0", tag="h")
        nc.sync.dma_start(out=h_cur[:], in_=h0t[:])


        rz_ring = const.tile([128, TB * 2 * W], f32)
        n_ring = const.tile([128, TB * W], f32)
        d_ring = const.tile([128, TB * W], f32)

        gi3 = gib[:].rearrange("p (m x) -> p m x", m=6)

        def gru_step(t, h_cur):
            s = t % TB
            pg = psG.tile([128, 3 * W], f32, name="pg", tag="pg")
            prz = pg[:, 0:2 * W]
            pn = pg[:, 2 * W:3 * W]
            # b_hh_n into pn
            nc.tensor.matmul(out=pn[:, 0:BL], lhsT=bhnsb[:, 0:128],
                             rhs=ones16[:], start=True, stop=False)
            nc.tensor.matmul(out=pn[:, BL:W], lhsT=bhnsb[:, 128:256],
                             rhs=ones16[:], start=True, stop=False)
            # h-side matmuls
            for m in range(4):
                for kc in range(2):
                    nc.tensor.matmul(
                        out=prz[:, m * BL:(m + 1) * BL],
                        lhsT=whhsb[:, (m * 2 + kc) * 128:(m * 2 + kc) * 128 + 128],
                        rhs=h_cur[:, kc * BL:(kc + 1) * BL],
                        start=(kc == 0), stop=(kc == 1),
                    )
            for j in range(2):
                for kc in range(2):
                    nc.tensor.matmul(
                        out=pn[:, j * BL:(j + 1) * BL],
                        lhsT=whhsb[:, ((4 + j) * 2 + kc) * 128:((4 + j) * 2 + kc) * 128 + 128],
                        rhs=h_cur[:, kc * BL:(kc + 1) * BL],
                        start=False, stop=(kc == 1),
                    )
            # rzin = gh (PSUM) + gi (strided view over 4 m-chunks)
            _g = gib[:]
            giv = bass.AP(tensor=_g.tensor, offset=_g.offset + t * BL,
                          ap=[_g.ap[0], [giw, 4], [1, BL]])
            rzin = gt.tile([128, 2 * W], f32, name="rzin", tag="rzin")
            nc.vector.tensor_tensor(out=rzin[:], in0=prz, in1=giv,
                                    op=mybir.AluOpType.add)
            trz = rz_ring[:, s * 2 * W:(s + 1) * 2 * W]
            nc.scalar.activation(out=trz, in_=rzin[:], func=AF.Tanh, bias=zc[:])
            # oz = 0.5 - 0.5*tz ; u = oz*mask
            ozt = gt.tile([128, W], f32, name="ozt", tag="ozt")
            nc.scalar.activation(out=ozt[:], in_=trz[:, W:2 * W], func=AF.Copy,
                                 scale=-0.5, bias=0.5)
            ut = gt.tile([128, W], f32, name="ut", tag="ut")
            nc.vector.tensor_tensor(out=ut[:], in0=ozt[:],
                                    in1=masks[:, t * W:(t + 1) * W],
                                    op=mybir.AluOpType.mult)
            # q = tr+1 ; m1 = q*pn ; m2 = m1 + gi_n
            qt = gt.tile([128, W], f32, name="qt", tag="qt")
            nc.vector.tensor_scalar(out=qt[:], in0=trz[:, 0:W], scalar1=1.0,
                                    scalar2=None, op0=mybir.AluOpType.add)
            m1 = gt.tile([128, W], f32, name="m1", tag="m1")
            nc.vector.tensor_tensor(out=m1[:], in0=qt[:], in1=pn[:],
                                    op=mybir.AluOpType.mult)
            m2 = gt.tile([128, W], f32, name="m2", tag="m2")
            nc.vector.tensor_tensor(out=m2[:], in0=m1[:],
                                    in1=gi3[:, 4:6, t * BL:(t + 1) * BL],
                                    op=mybir.AluOpType.add)
            nt = n_ring[:, s * W:(s + 1) * W]
            nc.scalar.activation(out=nt, in_=m2[:], func=AF.Tanh, bias=zc[:])
            dt_ = d_ring[:, s * W:(s + 1) * W]
            nc.vector.tensor_tensor(out=dt_, in0=nt, in1=h_cur[:],
                                    op=mybir.AluOpType.subtract)
            et = gt.tile([128, W], f32, name="et", tag="et")
            nc.vector.tensor_tensor(out=et[:], in0=ut[:], in1=dt_,
                                    op=mybir.AluOpType.mult)
            h_new = hp.tile([128, W], f32, name=f"h{t + 1}", tag="h")
            nc.vector.tensor_tensor(out=h_new[:], in0=h_cur[:], in1=et[:],
                                    op=mybir.AluOpType.add)
            return h_new

        def flush(i_blk, nsl):
            # z = 0.5 + 0.5*tz over the block; zd; hnew = n - zd; out = m*hnew
            _a = rz_ring[:]
            zv = bass.AP(tensor=_a.tensor, offset=_a.offset + W,
                         ap=[_a.ap[0], [2 * W, nsl], [1, W]])
            zz = gt.tile([128, TB * W], f32, name="zz", tag="zz")
            nc.vector.tensor_scalar(out=zz[:, :nsl * W], in0=zv, scalar1=0.5,
                                    scalar2=0.5, op0=mybir.AluOpType.mult,
                                    op1=mybir.AluOpType.add)
            zd = gt.tile([128, TB * W], f32, name="zd", tag="zd")
            nc.vector.tensor_tensor(out=zd[:, :nsl * W], in0=zz[:, :nsl * W],
                                    in1=d_ring[:, :nsl * W], op=mybir.AluOpType.mult)
            hn = gt.tile([128, TB * W], f32, name="hn", tag="hn")
            nc.vector.tensor_tensor(out=hn[:, :nsl * W], in0=n_ring[:, :nsl * W],
                                    in1=zd[:, :nsl * W], op=mybir.AluOpType.subtract)
            o0 = i_blk * TB * W
            nc.vector.tensor_tensor(out=osb[:, o0:o0 + nsl * W], in0=hn[:, :nsl * W],
                                    in1=masks[:, o0:o0 + nsl * W],
                                    op=mybir.AluOpType.mult)

        stageA(0)
        if nb > 1:
            stageA(1)
        for i in range(nb):
            t0 = i * TB
            tend = min(t0 + TB, nsteps)
            for t in range(t0, tend):
                h_cur = gru_step(t, h_cur)
            flush(i, tend - t0)
            if i + 2 < nb:
                stageA(i + 2)

        if dbg:
            nc.sync.dma_start(out=dgi[:], in_=gib[:])
            nc.sync.dma_start(out=drz[:], in_=rz_ring[:])
            nc.sync.dma_start(out=dnr[:], in_=n_ring[:])
            nc.sync.dma_start(out=ddr[:], in_=d_ring[:])
            mkf = const.tile([128, nsteps * W], f32)
            nc.vector.tensor_copy(out=mkf[:], in_=masks[:])
            nc.sync.dma_start(out=dmk[:, :nsteps * W], in_=mkf[:])
        nc.sync.dma_start(out=outT[:, :nsteps * W], in_=osb[:, :nsteps * W])
        if nsteps < T:
            zz0 = const.tile([128, (T - nsteps) * W], f32)
            nc.vector.memset(zz0[:], 0.0)
            nc.sync.dma_start(out=outT[:, nsteps * W:], in_=zz0[:])
        nc.sync.dma_start(out=hlast[:], in_=h_cur[:])

    nc.finalize()
    return nc


def _get_nc(nb, nsteps):
    key = (nb, nsteps)
    if key not in _CACHE:
        _CACHE[key] = _build(nb, nsteps)
    return _CACHE[key]


def _prep_core(c, baskets, dow, hour, days2next, lengths, h0):
    u0, u1 = c * BL, (c + 1) * BL
    bk = baskets[u0:u1]                      # [16, 100, 25]
    # t-blocked gather indices, slot order s = ((dt*16+b)*25+k)
    bkp = np.zeros((TP, BL, K), np.int64)
    bkp[:T] = bk.transpose(1, 0, 2)
    idxg = np.zeros((NB_FULL, 128, NPAD // 16), np.int16)
    idxc = np.zeros((NB_FULL, 128, K), np.int32)
    for i in range(NB_FULL):
        blk = bkp[i * TB:(i + 1) * TB].reshape(128, K)
        idxc[i] = blk.astype(np.int32)
        flat = np.concatenate([(blk.reshape(-1) - VHALF),
                               np.zeros(NPAD - NIDX, np.int64)])
        idxg[i] = np.tile(flat.astype(np.int16).reshape(NPAD // 16, 16).T, (8, 1))
    hour_t = np.ascontiguousarray(hour[u0:u1].T).astype(np.float32)
    dow_t = np.ascontiguousarray(dow[u0:u1].T).astype(np.float32)
    d2n_t = np.ascontiguousarray(days2next[u0:u1].T).astype(np.float32)
    lens = lengths[u0:u1].astype(np.float32).reshape(BL, 1)
    h0c = h0[0, u0:u1]                       # [16, 256]
    h0t = np.ascontiguousarray(
        h0c.reshape(BL, 2, 128).transpose(2, 1, 0).reshape(128, 2 * BL)
    ).astype(np.float32)
    return dict(idxg=idxg, idxc=idxc, hourt=hour_t, dowt=dow_t, d2nt=d2n_t,
                lens=lens, h0t=h0t)


def kernel(baskets, dow, hour, days2next, lengths, h0, enc_weight,
           W_ih, W_hh, b_ih, b_hh):
    from concourse.bass_utils import run_bass_kernel_spmd

    nb = int(os.environ.get("KNB", str(NB_FULL)))
    nsteps = min(T, nb * TB)
    nc = _get_nc(nb, nsteps)

    tab = np.asarray(enc_weight).astype(BF16)
    wihT = np.asarray(W_ih).T.copy()          # [266, 768]
    whhT = np.asarray(W_hh).T.copy()          # [256, 768]
    wihT[:, :2 * H] *= 0.5
    whhT *= 0.5
    wih_b = wihT.astype(np.float32)
    whh_b = whhT.astype(np.float32)
    brz_v = 0.5 * (np.asarray(b_ih)[:2 * H] + np.asarray(b_hh)[:2 * H])
    brz_np = np.ascontiguousarray(brz_v.reshape(4, 128).T).astype(np.float32)
    bn_np = np.ascontiguousarray(
        np.asarray(b_ih)[2 * H:].reshape(2, 128).T
    ).astype(np.float32)
    bhn_np = (0.5 * np.asarray(b_hh)[2 * H:]).reshape(1, H).astype(np.float32)
    iota_np = np.tile(np.arange(TP, dtype=np.float32), (BL, 1))
    identbf = np.eye(128, dtype=BF16)
    identf = np.eye(128, dtype=np.float32)

    shared = dict(tab=tab, wih=wih_b, whh=whh_b, brz=brz_np, bn=bn_np,
                  bhn=bhn_np, iotain=iota_np, identbf=identbf, identf=identf)
    in_maps = []
    for c in range(NCORES):
        m = _prep_core(c, baskets, dow, hour, days2next, lengths, h0)
        m["idxg"] = m["idxg"][:nb]
        m["idxc"] = m["idxc"][:nb]
        m.update(shared)
        in_maps.append(m)

    trace = os.environ.get("KTRACE", "0") == "1"
    res = run_bass_kernel_spmd(nc, in_maps, list(range(NCORES)), trace=trace)
    if trace:
        print("HW exec time:", res.exec_time_ns, "ns")
        if res.instructions_and_trace:
            print("trace:", res.instructions_and_trace[1])
        print("pjson:", res.profile_json)

    dyn = np.zeros((B, T, H), np.float32)
    hu = np.zeros((1, B, H), np.float32)
    for c in range(NCORES):
        o = res.results[c]["outT"]            # [128, T*2*BL]
        a = o.reshape(128, T, 2, BL).transpose(3, 1, 2, 0).reshape(BL, T, H)
        dyn[c * BL:(c + 1) * BL] = a
        hl = res.results[c]["hlast"]          # [128, 2*BL]
        hu[0, c * BL:(c + 1) * BL] = hl.reshape(128, 2, BL).transpose(2, 1, 0).reshape(BL, H)
    return dyn, hu


# revision 11
# speedup vs baseline: 1.6213x; 1.6213x over previous
"""Trainium2 Bass kernel for nn_DRModel_52630529245544.

Data-parallel over users (B=128 -> 16/core x 8 cores).
Per core:
  - basket embedding gather via GPSIMD dma_gather (transpose=True, int16
    indices biased around the table midpoint so signed int16 spans the
    50002-row vocab; +128 positive-sentinel tail indices)
  - masked mean-pool via DVE reduce (padding row 0 is all zeros)
  - x-side GRU projection precomputed for all timesteps (bf16 matmuls)
  - sequential GRU over T=100 in hidden-transposed layout [H in
    partitions, batch in free dim]; sigmoid computed via tanh identity
    (0.5*tanh(x/2)+0.5, folded into weights) so the ACT engine never
    reloads activation tables inside the loop.
"""
import os
import sys
import time

sys.path.insert(0, "/opt/trn_rl_repo")
sys.path.insert(0, os.path.dirname(os.path.abspath(__file__)))

import numpy as np
import ml_dtypes

BF16 = ml_dtypes.bfloat16

# problem constants (hardcoded per spec)
B, T, K, H, V = 128, 100, 25, 256, 50002
NCORES = 8
BL = B // NCORES          # 16 users per core
F = H + 10                # 266
G3 = 3 * H                # 768
TB = 8                    # timesteps per gather tile
NB_FULL = 13              # ceil(104/8); T padded to 104
TP = TB * NB_FULL         # 104
NIDX = 128 * K            # 3200 real slots per tile
NPAD = NIDX + 128         # +128 positive-sentinel tail
GIW = NB_FULL * 128       # 1664 columns per m-chunk in GI'
VHALF = V // 2            # 25001

_CACHE = {}


def _build(nb, nsteps):
    from tc_common import TC2
    import concourse.bass as bass
    import concourse.bacc as bacc
    from concourse import mybir

    nc = bacc.Bacc("TRN2")
    f32, bf16, i32, i16 = (
        mybir.dt.float32, mybir.dt.bfloat16, mybir.dt.int32, mybir.dt.int16,
    )
    AF = mybir.ActivationFunctionType

    # ---- I/O ----
    tab = nc.declare_dram_parameter("tab", [V, H], bf16, isOutput=False)
    idxg = nc.declare_dram_parameter("idxg", [nb, 128, NPAD // 16], i16, isOutput=False)
    idxc = nc.declare_dram_parameter("idxc", [nb, 128, K], i32, isOutput=False)
    wih = nc.declare_dram_parameter("wih", [F, G3], bf16, isOutput=False)
    whh = nc.declare_dram_parameter("whh", [H, G3], bf16, isOutput=False)
    brz = nc.declare_dram_parameter("brz", [128, 4], f32, isOutput=False)
    bn = nc.declare_dram_parameter("bn", [128, 2], f32, isOutput=False)
    bhn = nc.declare_dram_parameter("bhn", [1, H], f32, isOutput=False)
    hourt = nc.declare_dram_parameter("hourt", [T, BL], f32, isOutput=False)
    dowt = nc.declare_dram_parameter("dowt", [T, BL], f32, isOutput=False)
    d2nt = nc.declare_dram_parameter("d2nt", [T, BL], f32, isOutput=False)
    lens = nc.declare_dram_parameter("lens", [BL, 1], f32, isOutput=False)
    iotain = nc.declare_dram_parameter("iotain", [BL, TP], f32, isOutput=False)
    identbf = nc.declare_dram_parameter("identbf", [128, 128], bf16, isOutput=False)
    identf = nc.declare_dram_parameter("identf", [128, 128], f32, isOutput=False)
    h0t = nc.declare_dram_parameter("h0t", [128, 2 * BL], f32, isOutput=False)
    dbg = os.environ.get("KDBG", "0") == "1"
    if dbg:
        dgi = nc.declare_dram_parameter("dgi", [128, 6 * nb * 128], f32, isOutput=True)
        dpb = nc.declare_dram_parameter("dpb", [nb, 128, 2 * 128], f32, isOutput=True)
        dmk = nc.declare_dram_parameter("dmk", [128, nb * TB * 2 * BL], f32, isOutput=True)
        drz = nc.declare_dram_parameter("drz", [128, TB * 4 * BL], f32, isOutput=True)
        dnr = nc.declare_dram_parameter("dnr", [128, TB * 2 * BL], f32, isOutput=True)
        ddr = nc.declare_dram_parameter("ddr", [128, TB * 2 * BL], f32, isOutput=True)
    outT = nc.declare_dram_parameter("outT", [128, T * 2 * BL], f32, isOutput=True)
    hlast = nc.declare_dram_parameter("hlast", [128, 2 * BL], f32, isOutput=True)

    W = 2 * BL  # 32: (hc, b) folded column width

    with TC2(nc) as tc, (
        tc.tile_pool(name="const", bufs=1)
    ) as const, (
        tc.tile_pool(name="embp", bufs=2)
    ) as embp, (
        tc.tile_pool(name="stage", bufs=3)
    ) as stage, (
        tc.tile_pool(name="psA", bufs=3, space="PSUM")
    ) as psA, (
        tc.tile_pool(name="psG", bufs=2, space="PSUM")
    ) as psG, (
        tc.tile_pool(name="hp", bufs=2)
    ) as hp, (
        tc.tile_pool(name="gt", bufs=2)
    ) as gt:
        # ---------- constants / weights ----------
        ibf = const.tile([128, 128], bf16)
        nc.sync.dma_start(out=ibf[:], in_=identbf[:])
        if32 = const.tile([128, 128], f32)
        nc.sync.dma_start(out=if32[:], in_=identf[:])
        ones1 = const.tile([1, 128], f32)
        nc.vector.memset(ones1[:], 1.0)
        ones16 = const.tile([1, BL], f32)
        nc.vector.memset(ones16[:], 1.0)
        zc = const.tile([128, 1], f32)
        nc.vector.memset(zc[:], 0.0)
        pih = const.tile([128, 1], f32)
        nc.vector.memset(pih[:], float(np.pi / 2))

        # W_ih^T tiles: k-chunks (128,128,10) x m-chunks(6): [128, 18*128] bf16
        wihsb = const.tile([128, 18 * 128], bf16)
        for kc, (k0, kn) in enumerate(((0, 128), (128, 128), (256, 10))):
            for m in range(6):
                nc.sync.dma_start(
                    out=wihsb[:kn, (kc * 6 + m) * 128:(kc * 6 + m) * 128 + 128],
                    in_=wih[k0:k0 + kn, m * 128:(m + 1) * 128],
                )
        whhsb = const.tile([128, 12 * 128], bf16)
        for kc in range(2):
            for m in range(6):
                nc.sync.dma_start(
                    out=whhsb[:, (m * 2 + kc) * 128:(m * 2 + kc) * 128 + 128],
                    in_=whh[kc * 128:(kc + 1) * 128, m * 128:(m + 1) * 128],
                )
        brzsb = const.tile([128, 4], f32)
        nc.sync.dma_start(out=brzsb[:], in_=brz[:])
        bnsb = const.tile([128, 2], f32)
        nc.sync.dma_start(out=bnsb[:], in_=bn[:])
        bhnsb = const.tile([1, H], f32)
        nc.sync.dma_start(out=bhnsb[:], in_=bhn[:])

        # ---------- extra features -> xt_extra [10, nb*128] bf16 ----------
        hsb = const.tile([T, BL], f32)
        nc.sync.dma_start(out=hsb[:], in_=hourt[:])
        dsb = const.tile([T, BL], f32)
        nc.sync.dma_start(out=dsb[:], in_=dowt[:])
        nsb = const.tile([T, BL], f32)
        nc.sync.dma_start(out=nsb[:], in_=d2nt[:])
        feat = const.tile([T, 10 * BL], f32)
        PI = float(np.pi)
        nc.scalar.activation(out=feat[:, 0 * BL:1 * BL], in_=hsb[:], func=AF.Sin,
                             scale=PI / 23.0, bias=zc[:T])
        nc.scalar.activation(out=feat[:, 1 * BL:2 * BL], in_=hsb[:], func=AF.Sin,
                             scale=-PI / 23.0, bias=pih[:T])
        nc.scalar.activation(out=feat[:, 2 * BL:3 * BL], in_=nsb[:], func=AF.Copy,
                             scale=0.2)
        for j in range(7):
            nc.vector.tensor_scalar(
                out=feat[:, (3 + j) * BL:(4 + j) * BL], in0=dsb[:],
                scalar1=float(j), scalar2=None, op0=mybir.AluOpType.is_equal,
            )
        featbf = const.tile([T, 10 * BL], bf16)
        nc.vector.tensor_copy(out=featbf[:], in_=feat[:])
        xte = const.tile([10, nb * 128], bf16)
        n_tcols = min(T, nb * TB)
        for fi in range(10):
            nc.sync.dma_start(
                out=xte[fi:fi + 1, :n_tcols * BL].rearrange("o (t b) -> o t b", b=BL),
                in_=featbf[:n_tcols, fi * BL:(fi + 1) * BL],
            )

        # ---------- masks [128, nsteps*W] bf16 ----------
        iot = const.tile([BL, TP], f32)
        nc.sync.dma_start(out=iot[:], in_=iotain[:])
        lsb = const.tile([BL, 1], f32)
        nc.sync.dma_start(out=lsb[:], in_=lens[:])
        mrow = const.tile([BL, TP], f32)
        nc.vector.tensor_scalar(out=mrow[:], in0=iot[:], scalar1=lsb[:, 0:1],
                                scalar2=None, op0=mybir.AluOpType.is_lt)
        mps = psA.tile([TP, BL], f32, name="mps", tag="mps", bufs=1)
        nc.tensor.matmul(out=mps[:], lhsT=mrow[:], rhs=if32[:BL, :BL],
                         is_transpose=True, start=True, stop=True)
        mT = const.tile([TP, BL], f32)
        nc.scalar.activation(out=mT[:], in_=mps[:], func=AF.Copy)
        mrow2 = const.tile([1, nsteps * W], f32)
        _m = mrow2[:]
        for hc in range(2):
            dst = bass.AP(tensor=_m.tensor, offset=_m.offset + hc * BL,
                          ap=[_m.ap[0], [W, nsteps], [1, BL]])
            nc.sync.dma_start(out=dst, in_=mT[:nsteps, :])
        masks = const.tile([128, nsteps * W], bf16)
        c0 = 0
        while c0 < nsteps * W:
            cw = min(512, nsteps * W - c0)
            mbp = psA.tile([128, 512], f32, name="mbp", tag="mps", bufs=1)
            nc.tensor.matmul(out=mbp[:, :cw], lhsT=ones1[:], rhs=mrow2[:, c0:c0 + cw],
                             start=True, stop=True)
            nc.scalar.activation(out=masks[:, c0:c0 + cw], in_=mbp[:, :cw], func=AF.Copy)
            c0 += cw

        # ---------- GI' [128, 6 * nb*128] bf16 ----------
        gib = const.tile([128, 6 * nb * 128], f32)
        giw = nb * 128

        # ---------- output accumulator ----------
        osb = const.tile([128, T * W], f32)

        # ---------- stage A ----------
        def _bcast2(bass, tile_):
            a = tile_[:]
            return bass.AP(tensor=a.tensor, offset=a.offset,
                           ap=[a.ap[0], [0, 2], [1, 128]])

        def stageA(i):
            ncols = 128 if (i < nb - 1 or nb * TB <= T) else (T - (nb - 1) * TB) * BL
            it = embp.tile([128, NPAD // 16], i16, name="it", tag="it")
            nc.sync.dma_start(out=it[:], in_=idxg[i])
            embT = embp.tile([128, 2, NPAD], bf16, name="embT", tag="embT")
            nc.gpsimd.dma_gather(
                out_ap=embT[:], in_ap=tab[VHALF:, :], idxs_ap=it[:],
                num_idxs=NPAD, num_idxs_reg=NPAD, elem_size=H,
                transpose=True, single_packet=False,
            )
            red = stage.tile([128, 2, 128], f32, name="red", tag="red")
            nc.vector.reduce_sum(
                out=red[:],
                in_=embT[:, :, :NIDX].rearrange("p c (b k) -> p c b k", k=K),
                axis=mybir.AxisListType.X,
            )
            # counts -> reciprocal
            ict = stage.tile([128, K], i32, name="ict", tag="ict")
            nc.sync.dma_start(out=ict[:], in_=idxc[i])
            c1 = stage.tile([128, K], f32, name="c1", tag="c1")
            nc.vector.tensor_scalar(out=c1[:], in0=ict[:], scalar1=0.5,
                                    scalar2=None, op0=mybir.AluOpType.is_gt)
            cs = stage.tile([128, 1], f32, name="cs", tag="cs")
            nc.vector.reduce_sum(out=cs[:], in_=c1[:], axis=mybir.AxisListType.X)
            nc.vector.tensor_scalar(out=cs[:], in0=cs[:], scalar1=1.0,
                                    scalar2=None, op0=mybir.AluOpType.max)
            rc = stage.tile([128, 1], f32, name="rc", tag="rc")
            nc.vector.reciprocal(out=rc[:], in_=cs[:])
            rcp = psA.tile([1, 128], f32, name="rcp", tag="rcp", bufs=1)
            nc.tensor.matmul(out=rcp[:], lhsT=rc[:], rhs=if32[:],
                             is_transpose=True, start=True, stop=True)
            rcr = stage.tile([1, 128], f32, name="rcr", tag="rcr")
            nc.scalar.activation(out=rcr[:], in_=rcp[:], func=AF.Copy)
            rcb = psA.tile([128, 128], f32, name="rcb", tag="rcb", bufs=1)
            nc.tensor.matmul(out=rcb[:], lhsT=ones1[:], rhs=rcr[:],
                             start=True, stop=True)
            pbf = stage.tile([128, 2, 128], bf16, name="pbf", tag="pbf")
            nc.vector.tensor_tensor(
                out=pbf[:], in0=red[:],
                in1=_bcast2(bass, rcb),
                op=mybir.AluOpType.mult,
            )
            if dbg:
                nc.sync.dma_start(out=dpb[i], in_=pbf[:].rearrange("p c x -> p (c x)"))
            # x-side matmuls
            for m in range(6):
                gp = psA.tile([128, 128], f32, name="gp", tag="gp", bufs=2)
                nc.tensor.matmul(out=gp[:, :ncols], lhsT=wihsb[:, m * 128:(m + 1) * 128],
                                 rhs=pbf[:, 0, :ncols], start=True, stop=False)
                nc.tensor.matmul(out=gp[:, :ncols], lhsT=wihsb[:, (6 + m) * 128:(7 + m) * 128],
                                 rhs=pbf[:, 1, :ncols], start=False, stop=False)
                nc.tensor.matmul(out=gp[:, :ncols], lhsT=wihsb[:10, (12 + m) * 128:(12 + m) * 128 + 128],
                                 rhs=xte[:10, i * 128:i * 128 + ncols], start=False, stop=True)
                bias_ap = brzsb[:, m:m + 1] if m < 4 else bnsb[:, m - 4:m - 3]
                nc.scalar.activation(out=gib[:, m * giw + i * 128: m * giw + i * 128 + ncols],
                                     in_=gp[:, :ncols], func=AF.Identity, bias=bias_ap)

        # ---------- GRU ----------
        h_cur = hp.tile([128, W], f32, name="h0", tag="h")
        nc.sync.dma_start(out=h_cur[:], in_=h0t[:])
        hbf_cur = hp.tile([128, W], bf16, name="hb0", tag="hb")
        nc.vector.tensor_copy(out=hbf_cur[:], in_=h_cur[:])


        rz_ring = const.tile([128, TB * 2 * W], f32)
        n_ring = const.tile([128, TB * W], f32)
        d_ring = const.tile([128, TB * W], f32)

        gi3 = gib[:].rearrange("p (m x) -> p m x", m=6)

        def gru_step(t, h_cur, hbf_cur):
            s = t % TB
            pg = psG.tile([128, 3 * W], f32, name="pg", tag="pg")
            prz = pg[:, 0:2 * W]
            pn = pg[:, 2 * W:3 * W]
            # b_hh_n into pn
            nc.tensor.matmul(out=pn[:, 0:BL], lhsT=bhnsb[:, 0:128],
                             rhs=ones16[:], start=True, stop=False)
            nc.tensor.matmul(out=pn[:, BL:W], lhsT=bhnsb[:, 128:256],
                             rhs=ones16[:], start=True, stop=False)
            # h-side matmuls
            for m in range(4):
                for kc in range(2):
                    nc.tensor.matmul(
                        out=prz[:, m * BL:(m + 1) * BL],
                        lhsT=whhsb[:, (m * 2 + kc) * 128:(m * 2 + kc) * 128 + 128],
                        rhs=hbf_cur[:, kc * BL:(kc + 1) * BL],
                        start=(kc == 0), stop=(kc == 1),
                    )
            for j in range(2):
                for kc in range(2):
                    nc.tensor.matmul(
                        out=pn[:, j * BL:(j + 1) * BL],
                        lhsT=whhsb[:, ((4 + j) * 2 + kc) * 128:((4 + j) * 2 + kc) * 128 + 128],
                        rhs=hbf_cur[:, kc * BL:(kc + 1) * BL],
                        start=False, stop=(kc == 1),
                    )
            # rzin = gh (PSUM) + gi (strided view over 4 m-chunks)
            _g = gib[:]
            giv = bass.AP(tensor=_g.tensor, offset=_g.offset + t * BL,
                          ap=[_g.ap[0], [giw, 4], [1, BL]])
            rzin = gt.tile([128, 2 * W], f32, name="rzin", tag="rzin")
            nc.vector.tensor_tensor(out=rzin[:], in0=prz, in1=giv,
                                    op=mybir.AluOpType.add)
            trz = rz_ring[:, s * 2 * W:(s + 1) * 2 * W]
            nc.scalar.activation(out=trz, in_=rzin[:], func=AF.Tanh, bias=zc[:])
            # oz = 0.5 - 0.5*tz ; u = oz*mask
            ozt = gt.tile([128, W], f32, name="ozt", tag="ozt")
            nc.scalar.activation(out=ozt[:], in_=trz[:, W:2 * W], func=AF.Copy,
                                 scale=-0.5, bias=0.5)
            ut = gt.tile([128, W], f32, name="ut", tag="ut")
            nc.vector.tensor_tensor(out=ut[:], in0=ozt[:],
                                    in1=masks[:, t * W:(t + 1) * W],
                                    op=mybir.AluOpType.mult)
            # q = tr+1 ; m1 = q*pn ; m2 = m1 + gi_n
            qt = gt.tile([128, W], f32, name="qt", tag="qt")
            nc.vector.tensor_scalar(out=qt[:], in0=trz[:, 0:W], scalar1=1.0,
                                    scalar2=None, op0=mybir.AluOpType.add)
            m1 = gt.tile([128, W], f32, name="m1", tag="m1")
            nc.vector.tensor_tensor(out=m1[:], in0=qt[:], in1=pn[:],
                                    op=mybir.AluOpType.mult)
            m2 = gt.tile([128, W], f32, name="m2", tag="m2")
            nc.vector.tensor_tensor(out=m2[:], in0=m1[:],
                                    in1=gi3[:, 4:6, t * BL:(t + 1) * BL],
                                    op=mybir.AluOpType.add)
            nt = n_ring[:, s * W:(s + 1) * W]
            nc.scalar.activation(out=nt, in_=m2[:], func=AF.Tanh, bias=zc[:])
            dt_ = d_ring[:, s * W:(s + 1) * W]
            nc.vector.tensor_tensor(out=dt_, in0=nt, in1=h_cur[:],
                                    op=mybir.AluOpType.subtract)
            et = gt.tile([128, W], f32, name="et", tag="et")
            nc.vector.tensor_tensor(out=et[:], in0=ut[:], in1=dt_,
                                    op=mybir.AluOpType.mult)
            h_new = hp.tile([128, W], f32, name=f"h{t + 1}", tag="h")
            nc.vector.tensor_tensor(out=h_new[:], in0=h_cur[:], in1=et[:],
                                    op=mybir.AluOpType.add)
            hbf_new = hp.tile([128, W], bf16, name=f"hb{t + 1}", tag="hb")
            nc.vector.tensor_copy(out=hbf_new[:], in_=h_new[:])
            return h_new, hbf_new

        def flush(i_blk, nsl):
            # z = 0.5 + 0.5*tz over the block; zd; hnew = n - zd; out = m*hnew
            _a = rz_ring[:]
            zv = bass.AP(tensor=_a.tensor, offset=_a.offset + W,
                         ap=[_a.ap[0], [2 * W, nsl], [1, W]])
            zz = gt.tile([128, TB * W], f32, name="zz", tag="zz")
            nc.vector.tensor_scalar(out=zz[:, :nsl * W], in0=zv, scalar1=0.5,
                                    scalar2=0.5, op0=mybir.AluOpType.mult,
                                    op1=mybir.AluOpType.add)
            zd = gt.tile([128, TB * W], f32, name="zd", tag="zd")
            nc.vector.tensor_tensor(out=zd[:, :nsl * W], in0=zz[:, :nsl * W],
                                    in1=d_ring[:, :nsl * W], op=mybir.AluOpType.mult)
            hn = gt.tile([128, TB * W], f32, name="hn", tag="hn")
            nc.vector.tensor_tensor(out=hn[:, :nsl * W], in0=n_ring[:, :nsl * W],
                                    in1=zd[:, :nsl * W], op=mybir.AluOpType.subtract)
            o0 = i_blk * TB * W
            nc.vector.tensor_tensor(out=osb[:, o0:o0 + nsl * W], in0=hn[:, :nsl * W],
                                    in1=masks[:, o0:o0 + nsl * W],
                                    op=mybir.AluOpType.mult)

        stageA(0)
        if nb > 1:
            stageA(1)
        for i in range(nb):
            t0 = i * TB
            tend = min(t0 + TB, nsteps)
            for t in range(t0, tend):
                h_cur, hbf_cur = gru_step(t, h_cur, hbf_cur)
            flush(i, tend - t0)
            if i + 2 < nb:
                stageA(i + 2)

        if dbg:
            nc.sync.dma_start(out=dgi[:], in_=gib[:])
            nc.sync.dma_start(out=drz[:], in_=rz_ring[:])
            nc.sync.dma_start(out=dnr[:], in_=n_ring[:])
            nc.sync.dma_start(out=ddr[:], in_=d_ring[:])
            mkf = const.tile([128, nsteps * W], f32)
            nc.vector.tensor_copy(out=mkf[:], in_=masks[:])
            nc.sync.dma_start(out=dmk[:, :nsteps * W], in_=mkf[:])
        nc.sync.dma_start(out=outT[:, :nsteps * W], in_=osb[:, :nsteps * W])
        if nsteps < T:
            zz0 = const.tile([128, (T - nsteps) * W], f32)
            nc.vector.memset(zz0[:], 0.0)
            nc.sync.dma_start(out=outT[:, nsteps * W:], in_=zz0[:])
        nc.sync.dma_start(out=hlast[:], in_=h_cur[:])

    nc.finalize()
    return nc


def _get_nc(nb, nsteps):
    key = (nb, nsteps)
    if key not in _CACHE:
        _CACHE[key] = _build(nb, nsteps)
    return _CACHE[key]


def _prep_core(c, baskets, dow, hour, days2next, lengths, h0):
    u0, u1 = c * BL, (c + 1) * BL
    bk = baskets[u0:u1]                      # [16, 100, 25]
    # t-blocked gather indices, slot order s = ((dt*16+b)*25+k)
    bkp = np.zeros((TP, BL, K), np.int64)
    bkp[:T] = bk.transpose(1, 0, 2)
    idxg = np.zeros((NB_FULL, 128, NPAD // 16), np.int16)
    idxc = np.zeros((NB_FULL, 128, K), np.int32)
    for i in range(NB_FULL):
        blk = bkp[i * TB:(i + 1) * TB].reshape(128, K)
        idxc[i] = blk.astype(np.int32)
        flat = np.concatenate([(blk.reshape(-1) - VHALF),
                               np.zeros(NPAD - NIDX, np.int64)])
        idxg[i] = np.tile(flat.astype(np.int16).reshape(NPAD // 16, 16).T, (8, 1))
    hour_t = np.ascontiguousarray(hour[u0:u1].T).astype(np.float32)
    dow_t = np.ascontiguousarray(dow[u0:u1].T).astype(np.float32)
    d2n_t = np.ascontiguousarray(days2next[u0:u1].T).astype(np.float32)
    lens = lengths[u0:u1].astype(np.float32).reshape(BL, 1)
    h0c = h0[0, u0:u1]                       # [16, 256]
    h0t = np.ascontiguousarray(
        h0c.reshape(BL, 2, 128).transpose(2, 1, 0).reshape(128, 2 * BL)
    ).astype(np.float32)
    return dict(idxg=idxg, idxc=idxc, hourt=hour_t, dowt=dow_t, d2nt=d2n_t,
                lens=lens, h0t=h0t)


def kernel(baskets, dow, hour, days2next, lengths, h0, enc_weight,
           W_ih, W_hh, b_ih, b_hh):
    from concourse.bass_utils import run_bass_kernel_spmd

    nb = int(os.environ.get("KNB", str(NB_FULL)))
    nsteps = min(T, nb * TB)
    nc = _get_nc(nb, nsteps)

    tab = np.asarray(enc_weight).astype(BF16)
    wihT = np.asarray(W_ih).T.copy()          # [266, 768]
    whhT = np.asarray(W_hh).T.copy()          # [256, 768]
    wihT[:, :2 * H] *= 0.5
    whhT *= 0.5
    wih_b = wihT.astype(BF16)
    whh_b = whhT.astype(BF16)
    brz_v = 0.5 * (np.asarray(b_ih)[:2 * H] + np.asarray(b_hh)[:2 * H])
    brz_np = np.ascontiguousarray(brz_v.reshape(4, 128).T).astype(np.float32)
    bn_np = np.ascontiguousarray(
        np.asarray(b_ih)[2 * H:].reshape(2, 128).T
    ).astype(np.float32)
    bhn_np = (0.5 * np.asarray(b_hh)[2 * H:]).reshape(1, H).astype(np.float32)
    iota_np = np.tile(np.arange(TP, dtype=np.float32), (BL, 1))
    identbf = np.eye(128, dtype=BF16)
    identf = np.eye(128, dtype=np.float32)

    shared = dict(tab=tab, wih=wih_b, whh=whh_b, brz=brz_np, bn=bn_np,
                  bhn=bhn_np, iotain=iota_np, identbf=identbf, identf=identf)
    in_maps = []
    for c in range(NCORES):
        m = _prep_core(c, baskets, dow, hour, days2next, lengths, h0)
        m["idxg"] = m["idxg"][:nb]
        m["idxc"] = m["idxc"][:nb]
        m.update(shared)
        in_maps.append(m)

    trace = os.environ.get("KTRACE", "0") == "1"
    res = run_bass_kernel_spmd(nc, in_maps, list(range(NCORES)), trace=trace)
    if trace:
        print("HW exec time:", res.exec_time_ns, "ns")
        if res.instructions_and_trace:
            print("trace:", res.instructions_and_trace[1])
        print("pjson:", res.profile_json)

    dyn = np.zeros((B, T, H), np.float32)
    hu = np.zeros((1, B, H), np.float32)
    for c in range(NCORES):
        o = res.results[c]["outT"]            # [128, T*2*BL]
        a = o.reshape(128, T, 2, BL).transpose(3, 1, 2, 0).reshape(BL, T, H)
        dyn[c * BL:(c + 1) * BL] = a
        hl = res.results[c]["hlast"]          # [128, 2*BL]
        hu[0, c * BL:(c + 1) * BL] = hl.reshape(128, 2, BL).transpose(2, 1, 0).reshape(BL, H)
    return dyn, hu


# revision 12
# speedup vs baseline: 1.7081x; 1.0535x over previous
"""Trainium2 Bass kernel for nn_DRModel_52630529245544.

Data-parallel over users (B=128 -> 16/core x 8 cores).
Per core:
  - basket embedding gather via GPSIMD dma_gather (transpose=True, int16
    indices biased around the table midpoint so signed int16 spans the
    50002-row vocab; +128 positive-sentinel tail indices)
  - masked mean-pool via DVE reduce (padding row 0 is all zeros)
  - x-side GRU projection precomputed for all timesteps (bf16 matmuls)
  - sequential GRU over T=100 in hidden-transposed layout [H in
    partitions, batch in free dim]; sigmoid computed via tanh identity
    (0.5*tanh(x/2)+0.5, folded into weights) so the ACT engine never
    reloads activation tables inside the loop.
"""
import os
import sys
import time

sys.path.insert(0, "/opt/trn_rl_repo")
sys.path.insert(0, os.path.dirname(os.path.abspath(__file__)))

import numpy as np
import ml_dtypes

BF16 = ml_dtypes.bfloat16

# problem constants (hardcoded per spec)
B, T, K, H, V = 128, 100, 25, 256, 50002
NCORES = 8
BL = B // NCORES          # 16 users per core
F = H + 10                # 266
G3 = 3 * H                # 768
TB = 8                    # timesteps per gather tile
NB_FULL = 13              # ceil(104/8); T padded to 104
TP = TB * NB_FULL         # 104
NIDX = 128 * K            # 3200 real slots per tile
NPAD = NIDX + 128         # +128 positive-sentinel tail
GIW = NB_FULL * 128       # 1664 columns per m-chunk in GI'
VHALF = V // 2            # 25001

_CACHE = {}


def _build(nb, nsteps):
    from tc_common import TC2
    import concourse.bass as bass
    import concourse.bacc as bacc
    from concourse import mybir

    nc = bacc.Bacc("TRN2")
    f32, bf16, i32, i16 = (
        mybir.dt.float32, mybir.dt.bfloat16, mybir.dt.int32, mybir.dt.int16,
    )
    AF = mybir.ActivationFunctionType

    # ---- I/O ----
    tab = nc.declare_dram_parameter("tab", [V, H], bf16, isOutput=False)
    idxg = nc.declare_dram_parameter("idxg", [nb, 128, NPAD // 16], i16, isOutput=False)
    idxc = nc.declare_dram_parameter("idxc", [nb, 128, K], i32, isOutput=False)
    wih = nc.declare_dram_parameter("wih", [F, G3], bf16, isOutput=False)
    whh = nc.declare_dram_parameter("whh", [H, G3], bf16, isOutput=False)
    brz = nc.declare_dram_parameter("brz", [128, 4], f32, isOutput=False)
    bn = nc.declare_dram_parameter("bn", [128, 2], f32, isOutput=False)
    bhn = nc.declare_dram_parameter("bhn", [1, H], f32, isOutput=False)
    hourt = nc.declare_dram_parameter("hourt", [T, BL], f32, isOutput=False)
    dowt = nc.declare_dram_parameter("dowt", [T, BL], f32, isOutput=False)
    d2nt = nc.declare_dram_parameter("d2nt", [T, BL], f32, isOutput=False)
    lens = nc.declare_dram_parameter("lens", [BL, 1], f32, isOutput=False)
    iotain = nc.declare_dram_parameter("iotain", [BL, TP], f32, isOutput=False)
    identbf = nc.declare_dram_parameter("identbf", [128, 128], bf16, isOutput=False)
    identf = nc.declare_dram_parameter("identf", [128, 128], f32, isOutput=False)
    h0t = nc.declare_dram_parameter("h0t", [128, 2 * BL], f32, isOutput=False)
    dbg = os.environ.get("KDBG", "0") == "1"
    if dbg:
        dgi = nc.declare_dram_parameter("dgi", [128, 6 * nb * 128], f32, isOutput=True)
        dpb = nc.declare_dram_parameter("dpb", [nb, 128, 2 * 128], f32, isOutput=True)
        dmk = nc.declare_dram_parameter("dmk", [128, nb * TB * 2 * BL], f32, isOutput=True)
        drz = nc.declare_dram_parameter("drz", [128, TB * 4 * BL], f32, isOutput=True)
        dnr = nc.declare_dram_parameter("dnr", [128, TB * 2 * BL], f32, isOutput=True)
        ddr = nc.declare_dram_parameter("ddr", [128, TB * 2 * BL], f32, isOutput=True)
    outT = nc.declare_dram_parameter("outT", [128, T * 2 * BL], f32, isOutput=True)
    hlast = nc.declare_dram_parameter("hlast", [128, 2 * BL], f32, isOutput=True)

    W = 2 * BL  # 32: (hc, b) folded column width

    with TC2(nc) as tc, (
        tc.tile_pool(name="const", bufs=1)
    ) as const, (
        tc.tile_pool(name="embp", bufs=2)
    ) as embp, (
        tc.tile_pool(name="stage", bufs=3)
    ) as stage, (
        tc.tile_pool(name="psA", bufs=3, space="PSUM")
    ) as psA, (
        tc.tile_pool(name="psG", bufs=2, space="PSUM")
    ) as psG, (
        tc.tile_pool(name="hp", bufs=2)
    ) as hp, (
        tc.tile_pool(name="gt", bufs=2)
    ) as gt:
        # ---------- constants / weights ----------
        ibf = const.tile([128, 128], bf16)
        nc.sync.dma_start(out=ibf[:], in_=identbf[:])
        if32 = const.tile([128, 128], f32)
        nc.sync.dma_start(out=if32[:], in_=identf[:])
        ones1 = const.tile([1, 128], f32)
        nc.vector.memset(ones1[:], 1.0)
        ones16 = const.tile([1, BL], f32)
        nc.vector.memset(ones16[:], 1.0)
        onesp = const.tile([128, 1], f32)
        nc.vector.memset(onesp[:], 1.0)
        zc = const.tile([128, 1], f32)
        nc.vector.memset(zc[:], 0.0)
        pih = const.tile([128, 1], f32)
        nc.vector.memset(pih[:], float(np.pi / 2))

        # W_ih^T tiles: k-chunks (128,128,10) x m-chunks(6): [128, 18*128] bf16
        wihsb = const.tile([128, 18 * 128], bf16)
        for kc, (k0, kn) in enumerate(((0, 128), (128, 128), (256, 10))):
            for m in range(6):
                nc.sync.dma_start(
                    out=wihsb[:kn, (kc * 6 + m) * 128:(kc * 6 + m) * 128 + 128],
                    in_=wih[k0:k0 + kn, m * 128:(m + 1) * 128],
                )
        whhsb = const.tile([128, 12 * 128], bf16)
        for kc in range(2):
            for m in range(6):
                nc.sync.dma_start(
                    out=whhsb[:, (m * 2 + kc) * 128:(m * 2 + kc) * 128 + 128],
                    in_=whh[kc * 128:(kc + 1) * 128, m * 128:(m + 1) * 128],
                )
        brzsb = const.tile([128, 4], f32)
        nc.sync.dma_start(out=brzsb[:], in_=brz[:])
        bnsb = const.tile([128, 2], f32)
        nc.sync.dma_start(out=bnsb[:], in_=bn[:])
        bhnsb = const.tile([1, H], f32)
        nc.sync.dma_start(out=bhnsb[:], in_=bhn[:])

        # ---------- extra features -> xt_extra [10, nb*128] bf16 ----------
        hsb = const.tile([T, BL], f32)
        nc.sync.dma_start(out=hsb[:], in_=hourt[:])
        dsb = const.tile([T, BL], f32)
        nc.sync.dma_start(out=dsb[:], in_=dowt[:])
        nsb = const.tile([T, BL], f32)
        nc.sync.dma_start(out=nsb[:], in_=d2nt[:])
        feat = const.tile([T, 10 * BL], f32)
        PI = float(np.pi)
        nc.scalar.activation(out=feat[:, 0 * BL:1 * BL], in_=hsb[:], func=AF.Sin,
                             scale=PI / 23.0, bias=zc[:T])
        nc.scalar.activation(out=feat[:, 1 * BL:2 * BL], in_=hsb[:], func=AF.Sin,
                             scale=-PI / 23.0, bias=pih[:T])
        nc.scalar.activation(out=feat[:, 2 * BL:3 * BL], in_=nsb[:], func=AF.Copy,
                             scale=0.2)
        for j in range(7):
            nc.vector.tensor_scalar(
                out=feat[:, (3 + j) * BL:(4 + j) * BL], in0=dsb[:],
                scalar1=float(j), scalar2=None, op0=mybir.AluOpType.is_equal,
            )
        featbf = const.tile([T, 10 * BL], bf16)
        nc.vector.tensor_copy(out=featbf[:], in_=feat[:])
        xte = const.tile([10, nb * 128], bf16)
        n_tcols = min(T, nb * TB)
        for fi in range(10):
            nc.sync.dma_start(
                out=xte[fi:fi + 1, :n_tcols * BL].rearrange("o (t b) -> o t b", b=BL),
                in_=featbf[:n_tcols, fi * BL:(fi + 1) * BL],
            )

        # ---------- masks [128, nsteps*W] bf16 ----------
        iot = const.tile([BL, TP], f32)
        nc.sync.dma_start(out=iot[:], in_=iotain[:])
        lsb = const.tile([BL, 1], f32)
        nc.sync.dma_start(out=lsb[:], in_=lens[:])
        mrow = const.tile([BL, TP], f32)
        nc.vector.tensor_scalar(out=mrow[:], in0=iot[:], scalar1=lsb[:, 0:1],
                                scalar2=None, op0=mybir.AluOpType.is_lt)
        mps = psA.tile([TP, BL], f32, name="mps", tag="mps", bufs=1)
        nc.tensor.matmul(out=mps[:], lhsT=mrow[:], rhs=if32[:BL, :BL],
                         is_transpose=True, start=True, stop=True)
        mT = const.tile([TP, BL], f32)
        nc.scalar.activation(out=mT[:], in_=mps[:], func=AF.Copy)
        mrow2 = const.tile([1, nsteps * W], f32)
        _m = mrow2[:]
        for hc in range(2):
            dst = bass.AP(tensor=_m.tensor, offset=_m.offset + hc * BL,
                          ap=[_m.ap[0], [W, nsteps], [1, BL]])
            nc.sync.dma_start(out=dst, in_=mT[:nsteps, :])
        masks = const.tile([128, nsteps * W], bf16)
        c0 = 0
        while c0 < nsteps * W:
            cw = min(512, nsteps * W - c0)
            mbp = psA.tile([128, 512], f32, name="mbp", tag="mps", bufs=1)
            nc.tensor.matmul(out=mbp[:, :cw], lhsT=ones1[:], rhs=mrow2[:, c0:c0 + cw],
                             start=True, stop=True)
            nc.scalar.activation(out=masks[:, c0:c0 + cw], in_=mbp[:, :cw], func=AF.Copy)
            c0 += cw

        # ---------- GI' [128, 6 * nb*128] bf16 ----------
        gib = const.tile([128, 6 * nb * 128], f32)
        giw = nb * 128

        # ---------- output accumulator ----------
        osb = const.tile([128, T * W], f32)

        # ---------- stage A ----------
        def _bcast2(bass, tile_):
            a = tile_[:]
            return bass.AP(tensor=a.tensor, offset=a.offset,
                           ap=[a.ap[0], [0, 2], [1, 128]])

        def stageA(i):
            ncols = 128 if (i < nb - 1 or nb * TB <= T) else (T - (nb - 1) * TB) * BL
            it = embp.tile([128, NPAD // 16], i16, name="it", tag="it")
            nc.sync.dma_start(out=it[:], in_=idxg[i])
            embT = embp.tile([128, 2, NPAD], bf16, name="embT", tag="embT")
            nc.gpsimd.dma_gather(
                out_ap=embT[:], in_ap=tab[VHALF:, :], idxs_ap=it[:],
                num_idxs=NPAD, num_idxs_reg=NPAD, elem_size=H,
                transpose=True, single_packet=False,
            )
            red = stage.tile([128, 2, 128], f32, name="red", tag="red")
            nc.vector.reduce_sum(
                out=red[:],
                in_=embT[:, :, :NIDX].rearrange("p c (b k) -> p c b k", k=K),
                axis=mybir.AxisListType.X,
            )
            # counts -> reciprocal
            ict = stage.tile([128, K], i32, name="ict", tag="ict")
            nc.sync.dma_start(out=ict[:], in_=idxc[i])
            c1 = stage.tile([128, K], f32, name="c1", tag="c1")
            nc.vector.tensor_scalar(out=c1[:], in0=ict[:], scalar1=0.5,
                                    scalar2=None, op0=mybir.AluOpType.is_gt)
            cs = stage.tile([128, 1], f32, name="cs", tag="cs")
            nc.vector.reduce_sum(out=cs[:], in_=c1[:], axis=mybir.AxisListType.X)
            nc.vector.tensor_scalar(out=cs[:], in0=cs[:], scalar1=1.0,
                                    scalar2=None, op0=mybir.AluOpType.max)
            rc = stage.tile([128, 1], f32, name="rc", tag="rc")
            nc.vector.reciprocal(out=rc[:], in_=cs[:])
            rcp = psA.tile([1, 128], f32, name="rcp", tag="rcp", bufs=1)
            nc.tensor.matmul(out=rcp[:], lhsT=rc[:], rhs=if32[:],
                             is_transpose=True, start=True, stop=True)
            rcr = stage.tile([1, 128], f32, name="rcr", tag="rcr")
            nc.scalar.activation(out=rcr[:], in_=rcp[:], func=AF.Copy)
            rcb = psA.tile([128, 128], f32, name="rcb", tag="rcb", bufs=1)
            nc.tensor.matmul(out=rcb[:], lhsT=ones1[:], rhs=rcr[:],
                             start=True, stop=True)
            pbf = stage.tile([128, 2, 128], bf16, name="pbf", tag="pbf")
            nc.vector.tensor_tensor(
                out=pbf[:], in0=red[:],
                in1=_bcast2(bass, rcb),
                op=mybir.AluOpType.mult,
            )
            if dbg:
                nc.sync.dma_start(out=dpb[i], in_=pbf[:].rearrange("p c x -> p (c x)"))
            # x-side matmuls
            for m in range(6):
                gp = psA.tile([128, 128], f32, name="gp", tag="gp", bufs=2)
                nc.tensor.matmul(out=gp[:, :ncols], lhsT=wihsb[:, m * 128:(m + 1) * 128],
                                 rhs=pbf[:, 0, :ncols], start=True, stop=False)
                nc.tensor.matmul(out=gp[:, :ncols], lhsT=wihsb[:, (6 + m) * 128:(7 + m) * 128],
                                 rhs=pbf[:, 1, :ncols], start=False, stop=False)
                nc.tensor.matmul(out=gp[:, :ncols], lhsT=wihsb[:10, (12 + m) * 128:(12 + m) * 128 + 128],
                                 rhs=xte[:10, i * 128:i * 128 + ncols], start=False, stop=True)
                bias_ap = brzsb[:, m:m + 1] if m < 4 else bnsb[:, m - 4:m - 3]
                nc.scalar.activation(out=gib[:, m * giw + i * 128: m * giw + i * 128 + ncols],
                                     in_=gp[:, :ncols], func=AF.Identity, bias=bias_ap)

        # ---------- GRU ----------
        h_cur = hp.tile([128, W], f32, name="h0", tag="h")
        nc.sync.dma_start(out=h_cur[:], in_=h0t[:])
        hbf_cur = hp.tile([128, W], bf16, name="hb0", tag="hb")
        nc.vector.tensor_copy(out=hbf_cur[:], in_=h_cur[:])


        rz_ring = const.tile([128, TB * 2 * W], f32)
        n_ring = const.tile([128, TB * W], f32)
        d_ring = const.tile([128, TB * W], f32)

        gi3 = gib[:].rearrange("p (m x) -> p m x", m=6)

        def gru_step(t, h_cur, hbf_cur):
            s = t % TB
            pg = psG.tile([128, 3 * W], f32, name="pg", tag="pg")
            prz = pg[:, 0:2 * W]
            pn = pg[:, 2 * W:3 * W]
            # h-side matmuls
            for m in range(4):
                for kc in range(2):
                    nc.tensor.matmul(
                        out=prz[:, m * BL:(m + 1) * BL],
                        lhsT=whhsb[:, (m * 2 + kc) * 128:(m * 2 + kc) * 128 + 128],
                        rhs=hbf_cur[:, kc * BL:(kc + 1) * BL],
                        start=(kc == 0), stop=(kc == 1),
                    )
            for j in range(2):
                for kc in range(2):
                    nc.tensor.matmul(
                        out=pn[:, j * BL:(j + 1) * BL],
                        lhsT=whhsb[:, ((4 + j) * 2 + kc) * 128:((4 + j) * 2 + kc) * 128 + 128],
                        rhs=hbf_cur[:, kc * BL:(kc + 1) * BL],
                        start=(kc == 0), stop=(kc == 1),
                    )
            # rzin = gh (PSUM) + gi (strided view over 4 m-chunks)
            _g = gib[:]
            giv = bass.AP(tensor=_g.tensor, offset=_g.offset + t * BL,
                          ap=[_g.ap[0], [giw, 4], [1, BL]])
            rzin = gt.tile([128, 2 * W], f32, name="rzin", tag="rzin")
            nc.vector.tensor_tensor(out=rzin[:], in0=prz, in1=giv,
                                    op=mybir.AluOpType.add)
            trz = rz_ring[:, s * 2 * W:(s + 1) * 2 * W]
            nc.scalar.activation(out=trz, in_=rzin[:], func=AF.Tanh, bias=zc[:])
            # oz = 0.5 - 0.5*tz ; u = oz*mask
            ozt = gt.tile([128, W], f32, name="ozt", tag="ozt")
            nc.scalar.activation(out=ozt[:], in_=trz[:, W:2 * W], func=AF.Copy,
                                 scale=-0.5, bias=0.5)
            ut = gt.tile([128, W], f32, name="ut", tag="ut")
            nc.vector.tensor_tensor(out=ut[:], in0=ozt[:],
                                    in1=masks[:, t * W:(t + 1) * W],
                                    op=mybir.AluOpType.mult)
            # q = tr+1 ; m1 = q*pn ; m2 = m1 + gi_n
            qt = gt.tile([128, W], f32, name="qt", tag="qt")
            nc.vector.tensor_tensor(out=qt[:], in0=trz[:, 0:W],
                                    in1=onesp[:].to_broadcast([128, W]),
                                    op=mybir.AluOpType.add)
            m1 = gt.tile([128, W], f32, name="m1", tag="m1")
            nc.vector.tensor_tensor(out=m1[:], in0=qt[:], in1=pn[:],
                                    op=mybir.AluOpType.mult)
            m2 = gt.tile([128, W], f32, name="m2", tag="m2")
            nc.vector.tensor_tensor(out=m2[:], in0=m1[:],
                                    in1=gi3[:, 4:6, t * BL:(t + 1) * BL],
                                    op=mybir.AluOpType.add)
            nt = n_ring[:, s * W:(s + 1) * W]
            nc.scalar.activation(out=nt, in_=m2[:], func=AF.Tanh, bias=zc[:])
            dt_ = d_ring[:, s * W:(s + 1) * W]
            nc.vector.tensor_tensor(out=dt_, in0=nt, in1=h_cur[:],
                                    op=mybir.AluOpType.subtract)
            et = gt.tile([128, W], f32, name="et", tag="et")
            nc.vector.tensor_tensor(out=et[:], in0=ut[:], in1=dt_,
                                    op=mybir.AluOpType.mult)
            h_new = hp.tile([128, W], f32, name=f"h{t + 1}", tag="h")
            nc.vector.tensor_tensor(out=h_new[:], in0=h_cur[:], in1=et[:],
                                    op=mybir.AluOpType.add)
            hbf_new = hp.tile([128, W], bf16, name=f"hb{t + 1}", tag="hb")
            nc.vector.tensor_tensor(out=hbf_new[:], in0=h_cur[:], in1=et[:],
                                    op=mybir.AluOpType.add)
            return h_new, hbf_new

        def flush(i_blk, nsl):
            # z = 0.5 + 0.5*tz over the block; zd; hnew = n - zd; out = m*hnew
            _a = rz_ring[:]
            zv = bass.AP(tensor=_a.tensor, offset=_a.offset + W,
                         ap=[_a.ap[0], [2 * W, nsl], [1, W]])
            zz = gt.tile([128, TB * W], f32, name="zz", tag="zz")
            nc.vector.tensor_scalar(out=zz[:, :nsl * W], in0=zv, scalar1=0.5,
                                    scalar2=0.5, op0=mybir.AluOpType.mult,
                                    op1=mybir.AluOpType.add)
            zd = gt.tile([128, TB * W], f32, name="zd", tag="zd")
            nc.vector.tensor_tensor(out=zd[:, :nsl * W], in0=zz[:, :nsl * W],
                                    in1=d_ring[:, :nsl * W], op=mybir.AluOpType.mult)
            hn = gt.tile([128, TB * W], f32, name="hn", tag="hn")
            nc.vector.tensor_tensor(out=hn[:, :nsl * W], in0=n_ring[:, :nsl * W],
                                    in1=zd[:, :nsl * W], op=mybir.AluOpType.subtract)
            o0 = i_blk * TB * W
            nc.vector.tensor_tensor(out=osb[:, o0:o0 + nsl * W], in0=hn[:, :nsl * W],
                                    in1=masks[:, o0:o0 + nsl * W],
                                    op=mybir.AluOpType.mult)

        stageA(0)
        if nb > 1:
            stageA(1)
        for i in range(nb):
            t0 = i * TB
            tend = min(t0 + TB, nsteps)
            for t in range(t0, tend):
                h_cur, hbf_cur = gru_step(t, h_cur, hbf_cur)
            flush(i, tend - t0)
            if i + 2 < nb:
                stageA(i + 2)

        if dbg:
            nc.sync.dma_start(out=dgi[:], in_=gib[:])
            nc.sync.dma_start(out=drz[:], in_=rz_ring[:])
            nc.sync.dma_start(out=dnr[:], in_=n_ring[:])
            nc.sync.dma_start(out=ddr[:], in_=d_ring[:])
            mkf = const.tile([128, nsteps * W], f32)
            nc.vector.tensor_copy(out=mkf[:], in_=masks[:])
            nc.sync.dma_start(out=dmk[:, :nsteps * W], in_=mkf[:])
        nc.sync.dma_start(out=outT[:, :nsteps * W], in_=osb[:, :nsteps * W])
        if nsteps < T:
            zz0 = const.tile([128, (T - nsteps) * W], f32)
            nc.vector.memset(zz0[:], 0.0)
            nc.sync.dma_start(out=outT[:, nsteps * W:], in_=zz0[:])
        nc.sync.dma_start(out=hlast[:], in_=h_cur[:])

    nc.finalize()
    return nc


def _get_nc(nb, nsteps):
    key = (nb, nsteps)
    if key not in _CACHE:
        _CACHE[key] = _build(nb, nsteps)
    return _CACHE[key]


def _prep_core(c, baskets, dow, hour, days2next, lengths, h0):
    u0, u1 = c * BL, (c + 1) * BL
    bk = baskets[u0:u1]                      # [16, 100, 25]
    # t-blocked gather indices, slot order s = ((dt*16+b)*25+k)
    bkp = np.zeros((TP, BL, K), np.int64)
    bkp[:T] = bk.transpose(1, 0, 2)
    idxg = np.zeros((NB_FULL, 128, NPAD // 16), np.int16)
    idxc = np.zeros((NB_FULL, 128, K), np.int32)
    for i in range(NB_FULL):
        blk = bkp[i * TB:(i + 1) * TB].reshape(128, K)
        idxc[i] = blk.astype(np.int32)
        flat = np.concatenate([(blk.reshape(-1) - VHALF),
                               np.zeros(NPAD - NIDX, np.int64)])
        idxg[i] = np.tile(flat.astype(np.int16).reshape(NPAD // 16, 16).T, (8, 1))
    hour_t = np.ascontiguousarray(hour[u0:u1].T).astype(np.float32)
    dow_t = np.ascontiguousarray(dow[u0:u1].T).astype(np.float32)
    d2n_t = np.ascontiguousarray(days2next[u0:u1].T).astype(np.float32)
    lens = lengths[u0:u1].astype(np.float32).reshape(BL, 1)
    h0c = h0[0, u0:u1]                       # [16, 256]
    h0t = np.ascontiguousarray(
        h0c.reshape(BL, 2, 128).transpose(2, 1, 0).reshape(128, 2 * BL)
    ).astype(np.float32)
    return dict(idxg=idxg, idxc=idxc, hourt=hour_t, dowt=dow_t, d2nt=d2n_t,
                lens=lens, h0t=h0t)


def kernel(baskets, dow, hour, days2next, lengths, h0, enc_weight,
           W_ih, W_hh, b_ih, b_hh):
    from concourse.bass_utils import run_bass_kernel_spmd

    nb = int(os.environ.get("KNB", str(NB_FULL)))
    nsteps = min(T, nb * TB)
    nc = _get_nc(nb, nsteps)

    tab = np.asarray(enc_weight).astype(BF16)
    wihT = np.asarray(W_ih).T.copy()          # [266, 768]
    whhT = np.asarray(W_hh).T.copy()          # [256, 768]
    wihT[:, :2 * H] *= 0.5
    whhT *= 0.5
    wih_b = wihT.astype(BF16)
    whh_b = whhT.astype(BF16)
    brz_v = 0.5 * (np.asarray(b_ih)[:2 * H] + np.asarray(b_hh)[:2 * H])
    brz_np = np.ascontiguousarray(brz_v.reshape(4, 128).T).astype(np.float32)
    bn_np = np.ascontiguousarray(
        np.asarray(b_ih)[2 * H:].reshape(2, 128).T
    ).astype(np.float32)
    bhn_np = (0.5 * np.asarray(b_hh)[2 * H:]).reshape(1, H).astype(np.float32)
    iota_np = np.tile(np.arange(TP, dtype=np.float32), (BL, 1))
    identbf = np.eye(128, dtype=BF16)
    identf = np.eye(128, dtype=np.float32)

    shared = dict(tab=tab, wih=wih_b, whh=whh_b, brz=brz_np, bn=bn_np,
                  bhn=bhn_np, iotain=iota_np, identbf=identbf, identf=identf)
    in_maps = []
    for c in range(NCORES):
        m = _prep_core(c, baskets, dow, hour, days2next, lengths, h0)
        m["idxg"] = m["idxg"][:nb]
        m["idxc"] = m["idxc"][:nb]
        m.update(shared)
        in_maps.append(m)

    trace = os.environ.get("KTRACE", "0") == "1"
    res = run_bass_kernel_spmd(nc, in_maps, list(range(NCORES)), trace=trace)
    if trace:
        print("HW exec time:", res.exec_time_ns, "ns")
        if res.instructions_and_trace:
            print("trace:", res.instructions_and_trace[1])
        print("pjson:", res.profile_json)

    dyn = np.zeros((B, T, H), np.float32)
    hu = np.zeros((1, B, H), np.float32)
    for c in range(NCORES):
        o = res.results[c]["outT"]            # [128, T*2*BL]
        a = o.reshape(128, T, 2, BL).transpose(3, 1, 2, 0).reshape(BL, T, H)
        dyn[c * BL:(c + 1) * BL] = a
        hl = res.results[c]["hlast"]          # [128, 2*BL]
        hu[0, c * BL:(c + 1) * BL] = hl.reshape(128, 2, BL).transpose(2, 1, 0).reshape(BL, H)
    return dyn, hu
